# revision 13
# baseline (speedup 1.0000x reference)
"""Trainium2 Bass kernel for nn_DMMRLoss — matmul-count-minimized design.

Per core: 8 streams = (r'2 x c4), S=36 patches/stream, 288 patches.
  conv1: FULL im2col (x,y,z gathered; K=54 rows, fp8) -> ONE matmul per
    (patch, stream-tile): 288 MMs of N=343, no accumulation, no psum chains.
  evac: relu+bias+dz-gather PSUM->SBUF bf16 (c1 layout [r'][dz][j][oz',y,x]
    so conv2 fuses (j,oz') into one AP dim), ACT/DVE alternating.
  conv2: 432 MMs of N=486 (27 offsets x 8 streams x 2 j-halves), single
    psum pass, all 8 banks resident.
  fc1: 108 MMs of N=144 (27 pos x 2 oc-halves x 2 v), [64,128] stationaries.
  fc2: 2 MMs; host applies fc2 bias + tanh + weighted mean.

Rationale: on this bass->walrus toolchain every matmul costs ~50ns of
serialized LDWEIGHTS+dispatch+semaphore regardless of N (measured), so
total matmuls (288+432+108+2) is the main driver.
"""
import sys

sys.path.insert(0, '/opt/trn_rl_repo')

import numpy as np
import ml_dtypes

import concourse.bacc as bacc
import concourse.mybir as mybir
import concourse.tile as tile
from concourse import bass_utils
from concourse.ap import AP


PATCH = 17
THRESH = 0.5
NCORES = 8
NRP = 2            # r' row-halves (stream dim 1)
NCG = 4            # col groups (stream dim 2)
S = 36             # patches per stream
NG = 6             # X DMA groups
JG = S // NG       # patches per DMA group (6)
AF = mybir.ActivationFunctionType

DT = mybir.dt.bfloat16
NPDT = ml_dtypes.bfloat16
DT8 = mybir.dt.float8e4
NPDT8 = ml_dtypes.float8_e4m3

KROWS = 54            # full im2col rows: 2ci * 3dz * 3dy * 3dx
C1P = 343             # conv1 out positions (7^3)
C1G = 441             # dz-gathered size: 3dz * 3oz' * 49
FXS = NCG * S * C1P   # x free size per partition-row: 49392
FC1 = NRP * 3 * S * 147  # c1 free: r' x dz x j x (oz',y,x): 31752
NOUT = 2 * 8 * 18     # 288 outputs per core (v2 x slot8 x j18)


def _ap(a, dims, off=0):
    return AP(tensor=a.tensor, offset=a.offset + off, ap=[list(d) for d in dims])


_cache = {}


def _build():
    if 'nc' in _cache:
        return _cache['nc']

    nc = bacc.Bacc("TRN2", target_bir_lowering=False, debug=False,
                   num_devices=NCORES)

    x_d = nc.dram_tensor("x", (NG, 128, NCG * JG * C1P), DT8,
                         kind="ExternalInput")
    w1_d = nc.dram_tensor("w1", (128, 128), DT8, kind="ExternalInput")
    w2_d = nc.dram_tensor("w2", (128, 27 * 128), DT, kind="ExternalInput")
    wf1_d = nc.dram_tensor("wf1", (128, 54 * 128), DT, kind="ExternalInput")
    wf2_d = nc.dram_tensor("wf2", (128, 2), DT, kind="ExternalInput")
    b1_d = nc.dram_tensor("b1", (128, 1), mybir.dt.float32, kind="ExternalInput")
    b2_d = nc.dram_tensor("b2", (128, 1), mybir.dt.float32, kind="ExternalInput")
    bf1_d = nc.dram_tensor("bf1", (128, 2), mybir.dt.float32, kind="ExternalInput")
    o_d = nc.dram_tensor("o", (1, NOUT), mybir.dt.float32, kind="ExternalOutput")

    with tile.TileContext(nc) as tc:
        with (
            tc.tile_pool(name="const", bufs=1) as cpool,
            tc.tile_pool(name="xin", bufs=1) as xpool,
            tc.tile_pool(name="c1", bufs=1) as c1pool,
            tc.tile_pool(name="cc", bufs=1) as ccpool,
            tc.tile_pool(name="fin", bufs=1) as fpool,
            tc.tile_pool(name="ps", bufs=4, space="PSUM") as pspool,
        ):
            w1 = cpool.tile([128, 128], DT8)
            dum = cpool.tile([64, 512], DT8)
            nc.vector.memset(dum[:], 0.0)
            w2 = cpool.tile([128, 27 * 128], DT)
            wf1 = cpool.tile([128, 54 * 128], DT)
            wf2 = cpool.tile([128, 2], DT)
            b1 = cpool.tile([128, 1], mybir.dt.float32)
            b2 = cpool.tile([128, 1], mybir.dt.float32)
            bf1 = cpool.tile([128, 2], mybir.dt.float32)

            x = xpool.tile([128, FXS], DT8)
            c1 = c1pool.tile([128, FC1], DT)
            cc = ccpool.tile([128, 27 * 8 * 18], DT)   # [pos27][slot8][j18]
            f1 = fpool.tile([128, 2 * NOUT], DT)
            out_sb = fpool.tile([1, NOUT], mybir.dt.float32)

            # --- DMAs ---
            nc.gpsimd.dma_start(w1[:], w1_d[:])
            nc.sync.dma_start(b1[:], b1_d[:])
            nc.sync.dma_start(b2[:], b2_d[:])
            nc.sync.dma_start(bf1[:], bf1_d[:])
            nc.sync.dma_start(wf2[:], wf2_d[:])
            for g in range(NG):
                eng = nc.sync if g % 2 == 0 else nc.gpsimd
                dstx = _ap(x[:], [[FXS, 128], [S * C1P, NCG], [1, JG * C1P]],
                           off=g * JG * C1P)
                srcx = _ap(x_d[g],
                           [[NCG * JG * C1P, 128], [JG * C1P, NCG],
                            [1, JG * C1P]])
                eng.dma_start(dstx, srcx)
            nc.gpsimd.dma_start(w2[:], w2_d[:])
            nc.gpsimd.dma_start(wf1[:], wf1_d[:])

            # --- PE warmup during X DMA wait ---
            warm = pspool.tile([128, 1024], mybir.dt.float32, tag="ps",
                               name="warm")
            for _ in range(16):
                nc.tensor.matmul(warm[0:32, 0:343], w1[0:54, 0:32],
                                 dum[0:54, 0:343], start=True, stop=True,
                                 tile_position=(0, 0))

            xr = x[:].rearrange("p (c j f) -> p c j f", c=NCG, j=S)

            # --- conv1: 2 patch-idx per psum tile; bank = jloc*2 + r' ---
            for grp in range(S // 2):
                pt0 = pspool.tile([128, 1024], mybir.dt.float32, tag="ps",
                                  name=f"p1a{grp}")
                pt1 = pspool.tile([128, 1024], mybir.dt.float32, tag="ps",
                                  name=f"p1b{grp}")
                pts = (pt0, pt1)
                for jl in range(2):
                    j = grp * 2 + jl
                    for rp in range(NRP):
                        for c in range(NCG):
                            nc.tensor.matmul(
                                pts[rp][32 * c:32 * c + 32,
                                        jl * 512:jl * 512 + 343],
                                w1[64 * rp:64 * rp + KROWS, 32 * c:32 * c + 32],
                                xr[64 * rp:64 * rp + KROWS, c, j, :],
                                start=True, stop=True,
                                tile_position=(64 * rp, 32 * c))
                # evac: per (rp, dz); ACT gates only rp0 tiles, DVE rp1
                for rp in range(NRP):
                    for dz in range(3):
                        src = _ap(pts[rp][:],
                                  [[1024, 128], [512, 2], [98, 3], [1, 49]],
                                  off=dz * 49)
                        dst = _ap(c1[:], [[FC1, 128], [147, 2], [1, 147]],
                                  off=rp * (3 * S * 147) + dz * (S * 147)
                                      + grp * 2 * 147)
                        if rp == 0:
                            nc.scalar.activation(dst, src, AF.Relu,
                                                 bias=b1[:, 0:1])
                        else:
                            nc.vector.tensor_scalar(
                                dst, src, b1[:, 0:1], 0.0,
                                op0=mybir.AluOpType.add,
                                op1=mybir.AluOpType.max)

            # --- conv2: 27 o x 8 streams x 2 j-halves, N=486 ---
            # stream (r', c); psum: p2a = c 0,1 ; p2b = c 2,3 ;
            # bank (c%2)*2 + jh ; partitions 64r' + co
            p2 = [pspool.tile([128, 1024], mybir.dt.float32, tag="ps",
                              name=f"p2_{cc_}") for cc_ in range(4)]
            for o in range(27):
                dz, dy, dx = o // 9, (o // 3) % 3, o % 3
                for c in range(NCG):
                    for rp in range(NRP):
                        for jh in range(2):
                            rhs = _ap(
                                c1[:],
                                [[FC1, 32], [49, 54], [14, 3], [2, 3]],
                                off=32 * c * FC1 + rp * (3 * S * 147)
                                    + dz * (S * 147) + jh * 18 * 147
                                    + dy * 7 + dx)
                            nc.tensor.matmul(
                                p2[c][64 * rp:64 * rp + 64,
                                      jh * 512:jh * 512 + 486],
                                w2[32 * c:32 * c + 32,
                                   o * 128 + 64 * rp:o * 128 + 64 * rp + 64],
                                rhs, start=(o == 0), stop=(o == 26),
                                tile_position=(32 * c, 64 * rp))
            # cc evac: [j][pos] -> [pos][slot][j] ; slot = c*2 + jh
            for c in range(NCG):
                for jh in range(2):
                    sl = c * 2 + jh
                    src = _ap(p2[c][:], [[1024, 128], [1, 27], [27, 18]],
                              off=jh * 512)
                    dst = _ap(cc[:], [[27 * 144, 128], [18, 27], [1, 18]],
                              off=sl * 486)
                    if jh == 0:
                        nc.scalar.activation(dst, src, AF.Relu,
                                             bias=b2[:, 0:1])
                    else:
                        nc.vector.tensor_scalar(
                            dst, src, b2[:, 0:1], 0.0,
                            op0=mybir.AluOpType.add, op1=mybir.AluOpType.max)

            # --- fc1: 27 pos x 2 oc-halves; separate banks per v ---
            psf0 = pspool.tile([128, 1024], mybir.dt.float32, tag="ps",
                               name="psf0")
            psf1 = pspool.tile([128, 1024], mybir.dt.float32, tag="ps",
                               name="psf1")
            psf = (psf0, psf1)
            for pos in range(27):
                for h in range(2):
                    ch = pos * 2 + h
                    for v in range(2):
                        rhsf = _ap(cc[:], [[27 * 144, 64], [486, 8], [1, 18]],
                                   off=64 * v * (27 * 144) + pos * 18)
                        nc.tensor.matmul(
                            psf[h][0:128, v * 512:v * 512 + 144],
                            wf1[64 * v:64 * v + 64, ch * 128:(ch + 1) * 128],
                            rhsf,
                            start=(pos == 0), stop=(pos == 26),
                            tile_position=(64 * v, 0))
            srcf0 = _ap(psf0[:], [[1024, 128], [512, 2], [1, 144]])
            dstf0 = _ap(f1[:], [[2 * NOUT, 128], [144, 2], [1, 144]])
            nc.scalar.activation(dstf0, srcf0, AF.Relu, bias=bf1[:, 0:1])
            srcf1 = _ap(psf1[:], [[1024, 128], [512, 2], [1, 144]])
            dstf1 = _ap(f1[:], [[2 * NOUT, 128], [144, 2], [1, 144]], off=NOUT)
            nc.vector.tensor_scalar(dstf1, srcf1, bf1[:, 1:2], 0.0,
                                    op0=mybir.AluOpType.add,
                                    op1=mybir.AluOpType.max)

            # --- fc2 (host applies bias + tanh) ---
            psf2 = pspool.tile([128, 1024], mybir.dt.float32, tag="ps",
                               name="psf2")
            for h in range(2):
                nc.tensor.matmul(psf2[0:1, 0:NOUT], wf2[:, h:h + 1],
                                 f1[:, h * NOUT:(h + 1) * NOUT],
                                 start=(h == 0), stop=(h == 1),
                                 tile_position=(0, 0))
            nc.scalar.copy(out_sb[:], psf2[0:1, 0:NOUT])
            nc.sync.dma_start(o_d[:], out_sb[:])

    nc.compile()
    _cache['nc'] = nc
    return nc


def _bbox(mask):
    zs = np.flatnonzero(mask.any(axis=(1, 2)))
    ys = np.flatnonzero(mask.any(axis=(0, 2)))
    xs = np.flatnonzero(mask.any(axis=(0, 1)))
    return (int(xs[0]), int(ys[0]), int(zs[0]),
            int(xs[-1]), int(ys[-1]), int(zs[-1]))


def _extract(vol, bbox):
    x0, y0, z0, x1, y1, z1 = bbox
    t = vol[0, 0, z0:z1, y0:y1, x0:x1]
    pads = []
    for d in t.shape:
        rr = d % PATCH
        p = (PATCH - rr) % PATCH
        pads.append((p // 2, p - p // 2))
    t = np.pad(t, pads)
    D, H, W = t.shape
    nD, nH, nW = D // PATCH, H // PATCH, W // PATCH
    p = t.reshape(nD, PATCH, nH, PATCH, nW, PATCH)
    return p.transpose(0, 2, 4, 1, 3, 5).reshape(-1, PATCH, PATCH, PATCH)


def kernel(source, target, conv1_w, conv1_b, conv2_w, conv2_b,
           fc1_w, fc1_b, fc2_w, fc2_b):
    source = np.asarray(source, np.float32)
    target = np.asarray(target, np.float32)
    conv1_w = np.asarray(conv1_w, np.float32)
    conv1_b = np.asarray(conv1_b, np.float32)
    conv2_w = np.asarray(conv2_w, np.float32)
    conv2_b = np.asarray(conv2_b, np.float32)
    fc1_w = np.asarray(fc1_w, np.float32)
    fc1_b = np.asarray(fc1_b, np.float32)
    fc2_w = np.asarray(fc2_w, np.float32)
    fc2_b = np.asarray(fc2_b, np.float32)

    bbox = _bbox(target[0, 0] > 0)
    fixed = _extract(target, bbox)
    moving = _extract(source, bbox)
    Np = fixed.shape[0]
    keep = ((fixed == 0).reshape(Np, -1).mean(axis=1) <= THRESH).astype(np.float32)

    Npad = NCORES * NRP * NCG * S   # 2304
    assert Np <= Npad

    nc = _build()

    # --- X: FULL im2col [54 rows=(ci,dz,dy,dx)] x [343=(oz,oy,ox)] ---
    P2 = np.zeros((Npad, 2, PATCH, PATCH, PATCH), np.float32)
    P2[:Np, 0] = fixed
    P2[:Np, 1] = moving
    s0, s1, s2, s3, s4 = P2.strides
    cols = np.lib.stride_tricks.as_strided(
        P2, (Npad, 2, 3, 3, 3, 7, 7, 7),
        (s0, s1, s2, s3, s4, 2 * s2, 2 * s3, 2 * s4))
    # patch p = ((core*2 + r')*4 + c)*S + j ; device [g][128=(r',64row)][c][jj][343]
    colsr = cols.reshape(NCORES, NRP, NCG, NG, JG, KROWS, C1P)
    ct = colsr.transpose(0, 3, 1, 5, 2, 4, 6)  # [core][g][rp][row54][c][jj][343]
    X8 = np.zeros((NCORES, NG, NRP, 64, NCG, JG, C1P), NPDT8)
    X8[:, :, :, :KROWS] = ct.astype(NPDT8)

    # --- weights ---
    w1t = conv1_w.transpose(1, 2, 3, 4, 0).reshape(KROWS, 32)  # (ci,dz,dy,dx),co
    W1 = np.zeros((2, 64, 4, 32), np.float32)   # [r'][row64][c][co]
    W1[:, :KROWS] = w1t[None, :, None, :]
    W1 = W1.reshape(128, 128).astype(NPDT8)

    w2t = conv2_w.transpose(1, 2, 3, 4, 0).reshape(32, 27, 64)  # ci,o,co
    W2 = np.zeros((4, 32, 27, 2, 64), np.float32)  # [c][ci][o][v][co]
    W2[:] = w2t[None, :, :, None, :]
    W2 = W2.reshape(128, 27 * 128).astype(NPDT)

    wf1t = fc1_w.reshape(2, 128, 64, 27)           # [h][oc][co][pos]
    A = wf1t.transpose(2, 3, 0, 1).reshape(64, 54 * 128)
    WF1 = np.concatenate([A, A], axis=0).astype(NPDT)

    WF2 = fc2_w.reshape(2, 128).T.copy().astype(NPDT)
    B1 = np.tile(conv1_b, 4).reshape(128, 1).astype(np.float32)
    B2 = np.tile(conv2_b, 2).reshape(128, 1).astype(np.float32)
    BF1 = fc1_b.reshape(2, 128).T.copy().astype(np.float32)

    in_maps = []
    for core in range(NCORES):
        in_maps.append({
            "x": np.ascontiguousarray(X8[core]).reshape(NG, 128,
                                                        NCG * JG * C1P),
            "w1": W1, "w2": W2, "wf1": WF1, "wf2": WF2,
            "b1": B1, "b2": B2, "bf1": BF1,
        })

    res = bass_utils.run_bass_kernel_spmd(nc, in_maps,
                                          core_ids=list(range(NCORES)))
    global _last_results
    _last_results = res

    # --- gather: out col = v*144 + slot*18 + jj ; slot=c*2+jh ---
    y = np.zeros(Npad, np.float32)
    o = np.stack([res.results[core]["o"][0] for core in range(NCORES)])
    ov = o.reshape(NCORES, 2, 8, 18)               # core, v=r', slot, jj
    for v in range(2):
        for sl in range(8):
            c = sl // 2
            jh = sl % 2
            base = ((v * NCG) + c) * S + jh * 18
            for core in range(NCORES):
                y[core * NRP * NCG * S + base:
                  core * NRP * NCG * S + base + 18] = ov[core, v, sl]

    yt = np.tanh(y + fc2_b[0])
    out = np.sum(yt[:Np] * keep) / np.sum(keep)
    return np.float32(out)
